# revision 15
# baseline (speedup 1.0000x reference)
"""BERT self-attention (B=16, T=512, C=768, H=12, D=64) on 8 trn2 NeuronCores.

Data-parallel over batch: each core gets 2 batches. Matmul operands are fp16
(11-bit mantissa, ~tf32-class precision, 1 cycle/row PE streaming, FWL weight
loads); all accumulation stays fp32 in PSUM. Per core:
  xT    = x transposed on the host during input prep (fp16, [C, M] layout).
  Q^T/K^T ([feature, token] layout, lhsT = W_attn tile) and V ([token, feature]
          layout with a per-head [V_h | ones] 65-column block, lhsT = xT tile).
  S^T   = K^T-as-lhsT matmul -> scores in [key, query] layout (K=64, head pairs
          packed in PE row groups via base-partition-64 slices).
  P     = exp(S/8 + mask) on ScalarE (mask is a per-partition bias in this
          layout), written as fp16.
  y^T   = lhsT=[V_h | ones] matmul -> unnormalized y^T plus softmax row-sums as
          an extra PSUM row; row-sums are collected per batch, inverted with a
          fast Newton-Raphson reciprocal on DVE, replicated across partitions
          by small PE matmuls, and applied with a DVE multiply.
  out   = y^T-as-lhsT matmul with W_proj + b_proj (fp32 result to DRAM).
Bias adds ride the PSUM->SBUF drain (scalar activation / DVE add). Output
stores and staging DMAs round-robin across the three DMA queues.
"""

import sys

sys.path.insert(0, "/opt/trn_rl_repo")

from contextlib import ExitStack

import numpy as np

B, T, C = 16, 512, 768
H, D = 12, 64
C3 = 3 * C
N_CORES = 8
BC = B // N_CORES           # batches per core
M = BC * T                  # tokens per core
KT = C // 128               # feature k-tiles (6)
TT = M // 128               # token tiles per core (8)
NQK = 2 * C // 128          # q+k feature n-tiles (12)
VW = H * 65                 # v tile width: per-head [V_h | ones] blocks
SCALE = 1.0 / np.sqrt(D)

_cache = {}


def _build():
    import concourse.bass as bass
    import concourse.tile as tile
    from concourse import bacc, mybir
    f32 = mybir.dt.float32
    f16 = mybir.dt.float16
    Exp = mybir.ActivationFunctionType.Exp
    Ident = mybir.ActivationFunctionType.Identity

    nc = bacc.Bacc("TRN2", target_bir_lowering=False, debug=False,
                   num_devices=N_CORES)
    x_d = nc.dram_tensor("x", [C, M], f16, kind="ExternalInput").ap()
    mask_d = nc.dram_tensor("mask", [BC, T], f32, kind="ExternalInput").ap()
    wa_d = nc.dram_tensor("w_attn", [C, C3], f16, kind="ExternalInput").ap()
    ba_d = nc.dram_tensor("b_attn", [1, C3], f16, kind="ExternalInput").ap()
    wp_d = nc.dram_tensor("w_proj", [C, C], f16, kind="ExternalInput").ap()
    bp_d = nc.dram_tensor("b_proj", [1, C], f16, kind="ExternalInput").ap()
    out_d = nc.dram_tensor("out", [M, C], f32, kind="ExternalOutput").ap()

    # scalar's queue is kept free of bulk DMA: the scalar engine owns the
    # PSUM drains + EXPs and queue entries would delay them (FIFO per engine)
    dmaq2 = [nc.sync, nc.gpsimd]
    dq_state = {"i": 0}

    def next_q():
        q = dmaq2[dq_state["i"] % 2]
        dq_state["i"] += 1
        return q

    with tile.TileContext(nc) as tc, ExitStack() as ctx:
        pp = ctx.enter_context(tc.tile_pool(name="pp", bufs=1))
        np_ = ctx.enter_context(tc.tile_pool(name="norm", bufs=4))
        ap_ = ctx.enter_context(tc.tile_pool(name="att", bufs=6))
        ps_mm = ctx.enter_context(tc.tile_pool(name="ps_mm", bufs=2, space="PSUM"))
        ps_s = ctx.enter_context(tc.tile_pool(name="ps_s", bufs=2, space="PSUM"))
        ps_y = ctx.enter_context(tc.tile_pool(name="ps_y", bufs=2, space="PSUM"))

        # --- prologue -------------------------------------------------
        # Dummy activation first: forces the activation-table const DMA +
        # ACT_TABLE_LOAD to the front of the queues (lazily emitted at first
        # use, it would otherwise sit behind all the bulk input DMAs and
        # stall the first PSUM drains).
        ones = pp.tile([1, 128], f16, tag="ones")
        nc.vector.memset(ones[:], 1.0)
        ones16 = pp.tile([128, 64], f16, tag="ones16")
        nc.vector.memset(ones16[:], 1.0)
        warm = pp.tile([1, 2], f32, tag="warm")
        nc.scalar.activation(warm[:], ones[0:1, 0:2], Exp, bias=0.0, scale=1.0)

        # DMA schedule (latency-bound: ~0.6us/DMA regardless of size, so the
        # 6 per-k W chunks are coalesced into single 3D-AP DMAs):
        # sync:   wa[:, 0:384] (first 3 QK n-tiles, all k), xT k=0,2,4,
        #         wa[:, 384:1536] (rest of QK), wp.
        # gpsimd: xT k=1,3,5, ba_qk, mask, ba, bp, wa[:, 1536:2304] (V).
        wa_all = pp.tile([128, KT * C3], f16, tag="wa")
        wa_t = [wa_all[:, k * C3:(k + 1) * C3] for k in range(KT)]
        wa_sb3 = wa_all.rearrange("p (k c) -> p k c", c=C3)
        wa_d3 = wa_d.rearrange("(k p) c -> p k c", p=128)
        xT = [pp.tile([128, M], f16, tag=f"xT{k}", name=f"xT{k}")
              for k in range(KT)]
        nc.sync.dma_start(wa_sb3[:, :, 0:384], wa_d3[:, :, 0:384])
        for k in (0, 2, 4):
            nc.sync.dma_start(xT[k][:], x_d[k * 128:(k + 1) * 128, :])
        for k in (1, 3, 5):
            nc.gpsimd.dma_start(xT[k][:], x_d[k * 128:(k + 1) * 128, :])
        ba_qk = pp.tile([128, NQK], f32, tag="ba_qk")
        nc.gpsimd.dma_start(
            ba_qk[:],
            ba_d[0, 0:2 * C].rearrange("(j p) -> p j", p=128))
        mask_sb = pp.tile([128, BC * 4], f32, tag="mask")
        nc.gpsimd.dma_start(
            mask_sb[:],
            mask_d.rearrange("a b -> (a b)").rearrange("(j p) -> p j", p=128))
        ba_t = pp.tile([1, C3], f16, tag="ba")
        nc.gpsimd.dma_start(ba_t[:], ba_d[:])
        bp_t = pp.tile([1, C], f16, tag="bp")
        nc.gpsimd.dma_start(bp_t[:], bp_d[:])
        nc.sync.dma_start(wa_sb3[:, :, 384:1152], wa_d3[:, :, 384:1152])
        nc.sync.dma_start(wa_sb3[:, :, 1152:1536], wa_d3[:, :, 1152:1536])
        nc.gpsimd.dma_start(wa_sb3[:, :, 1536:2304], wa_d3[:, :, 1536:2304])
        wp_all = pp.tile([128, KT * C], f16, tag="wp")
        wp_t = [wp_all[:, k * C:(k + 1) * C] for k in range(KT)]
        nc.sync.dma_start(
            wp_all.rearrange("p (k c) -> p k c", c=C),
            wp_d.rearrange("(k p) c -> p k c", p=128))

        # bias rows replicated across partitions via K=1 matmuls
        ba_v_rep = pp.tile([128, C], f32, tag="ba_v_rep")
        bp_rep = pp.tile([128, C], f32, tag="bp_rep")

        def brep_emit():
            for lo, w in ((0, 512), (512, 256)):
                p = ps_mm.tile([128, 512], f32, tag="mm", name=f"brep{lo}")
                nc.tensor.matmul(
                    p[:, :w], ones[0:1, 0:128],
                    ba_t[0:1, 2 * C + lo:2 * C + lo + w], start=True, stop=True)
                nc.vector.tensor_copy(ba_v_rep[:, lo:lo + w], p[:, :w])
                p2 = ps_mm.tile([128, 512], f32, tag="mm", name=f"bprep{lo}")
                nc.tensor.matmul(
                    p2[:, :w], ones[0:1, 0:128],
                    bp_t[0:1, lo:lo + w], start=True, stop=True)
                nc.vector.tensor_copy(bp_rep[:, lo:lo + w], p2[:, :w])

        v_t = [pp.tile([128, VW], f16, tag=f"v{t}", name=f"v{t}")
               for t in range(TT)]
        qkT = [pp.tile([128, M], f16, tag=f"qk{n}", name=f"qk{n}")
               for n in range(NQK)]
        yT_t = [pp.tile([128, M], f16, tag=f"yT{c}", name=f"yT{c}")
                for c in range(KT)]
        for t in range(TT):
            nc.vector.memset(
                v_t[t].rearrange("p (h c) -> p h c", c=65)[:, :, D:D + 1], 1.0)

        def qkv_chain(b, i):
            """i in [0, 20): 12 QK n-tiles then 8 V half-tiles."""
            bcol = b * T
            if i < NQK:
                n = i
                p = ps_mm.tile([128, 512], f32, tag="mm", name=f"mm{b}_{i}")
                for k in range(KT):
                    nc.tensor.matmul(
                        p[:],
                        wa_t[k][:, n * 128:(n + 1) * 128],
                        xT[k][:, bcol:bcol + T],
                        start=(k == 0), stop=(k == KT - 1))
                # drain PSUM->SBUF fused with bias add; split scalar/DVE
                if n % 2 == 0:
                    nc.scalar.activation(
                        qkT[n][:, bcol:bcol + T], p[:], Ident,
                        bias=ba_qk[:, n:n + 1], scale=1.0)
                else:
                    nc.vector.tensor_scalar_add(
                        qkT[n][:, bcol:bcol + T], p[:], ba_qk[:, n:n + 1])
            else:
                j = i - NQK
                t = b * 4 + j // 2
                lo, w = ((0, 512), (512, 256))[j % 2]
                p = ps_mm.tile([128, 512], f32, tag="mm", name=f"mm{b}_{i}")
                for k in range(KT):
                    nc.tensor.matmul(
                        p[:, :w],
                        xT[k][:, t * 128:(t + 1) * 128],
                        wa_t[k][:, 2 * C + lo:2 * C + lo + w],
                        start=(k == 0), stop=(k == KT - 1))
                h0 = lo // D
                nc.vector.tensor_tensor(
                    out=v_t[t].rearrange("p (h c) -> p h c", c=65)
                        [:, h0:h0 + w // D, 0:D],
                    in0=p[:, :w].rearrange("p (h c) -> p h c", c=D),
                    in1=ba_v_rep[:, lo:lo + w].rearrange(
                        "p (h c) -> p h c", c=D),
                    op=mybir.AluOpType.add)

        yun_all = {}
        r_tiles = {}

        def attention_hp(b, hp):
            bcol = b * T
            if hp % 2 == 0:
                rt = np_.tile([97, 512], f32, tag="r_all", bufs=3,
                              name=f"r_all{b}_{hp // 2}")
                nc.vector.memset(rt[:], 1.0)
                r_tiles[(b, hp // 2)] = rt
            e_tiles = []
            for kt in range(4):
                ps = ps_s.tile([128, 1024], f32)
                for sub in range(2):
                    r0 = 64 * sub
                    nc.tensor.matmul(
                        ps[:, sub * 512:sub * 512 + 512],
                        qkT[6 + hp][r0:r0 + D,
                                    bcol + kt * 128:bcol + (kt + 1) * 128],
                        qkT[hp][r0:r0 + D, bcol:bcol + T],
                        start=True, stop=True)
                e = ap_.tile([128, 1024], f16, tag="e")
                nc.scalar.activation(
                    e[:], ps[:], Exp,
                    bias=mask_sb[:, b * 4 + kt:b * 4 + kt + 1],
                    scale=float(SCALE))
                e_tiles.append(e)
            for sub in range(2):
                h = 2 * hp + sub
                py = ps_y.tile([128, 512], f32)
                for kt in range(4):
                    nc.tensor.matmul(
                        py[0:65, :],
                        v_t[b * 4 + kt][:, 65 * h:65 * (h + 1)],
                        e_tiles[kt][:, sub * 512:sub * 512 + 512],
                        start=(kt == 0), stop=(kt == 3))
                yun = np_.tile([64, 512], f16, tag="yun", bufs=14,
                               name=f"yun{b}_{h}")
                nc.vector.tensor_copy(yun[:], py[0:D, :])
                rs = np_.tile([D + 1, 512], f32, tag="rstage")
                nc.vector.tensor_copy(rs[D:D + 1, :], py[D:D + 1, :])
                next_q().dma_start(
                    r_tiles[(b, hp // 2)][32 * (h % 4):32 * (h % 4) + 1, :],
                    rs[D:D + 1, :])
                yun_all[(b, h)] = yun

        recip_tiles = {}

        def norm_recip(b, grp):
            recip = np_.tile([97, 512], f32, tag="recip", bufs=3)
            nc.vector.reciprocal_approx_fast(recip[:], r_tiles[(b, grp)][:])
            recip16 = np_.tile([97, 512], f16, tag="recip16", bufs=3)
            nc.vector.tensor_copy(recip16[:], recip[:])
            recip_tiles[(b, grp)] = recip16

        def norm_apply(b, grp):
            bcol = b * T
            recip16 = recip_tiles[(b, grp)]
            # odd heads first: their cross-partition staging DMA latency is
            # hidden behind the even heads' direct multiplies
            for h in sorted(range(4 * grp, 4 * grp + 4),
                            key=lambda h: -(h % 2)):
                nt, r0 = h // 2, 64 * (h % 2)
                j = 32 * (h % 4)
                rep = ps_y.tile([128, 512], f32, tag="py", name=f"rep{b}_{h}")
                nc.tensor.matmul(
                    rep[0:64, :], ones16[j:j + 1, :], recip16[j:j + 1, :],
                    start=True, stop=True, tile_position=(j, 0))
                dst = yT_t[nt][r0:r0 + D, bcol:bcol + T]
                if r0 == 0:
                    nc.vector.tensor_mul(dst, yun_all[(b, h)][:], rep[0:64, :])
                else:
                    st = np_.tile([64, 512], f16, tag="stage")
                    nc.vector.tensor_mul(st[:], yun_all[(b, h)][:],
                                         rep[0:64, :])
                    next_q().dma_start(dst, st[:])

        pj_part = {}
        fin_state = {"i": 0}

        def proj_chunk(b, i, ks=0, ke=KT, partial=False, final=False):
            t = b * 4 + i // 2
            lo, w = ((0, 512), (512, 256))[i % 2]
            p = ps_mm.tile([128, 512], f32, tag="mm", name=f"pj{b}_{i}_{ks}")
            for k in range(ks, ke):
                nc.tensor.matmul(
                    p[:, :w],
                    yT_t[k][:, t * 128:(t + 1) * 128],
                    wp_t[k][:, lo:lo + w],
                    start=(k == ks), stop=(k == ke - 1))
            if partial:
                pt = np_.tile([128, 512], f32, tag="pjpart", bufs=8,
                              name=f"pjpart{i}")
                nc.vector.tensor_tensor(
                    out=pt[:, :w], in0=p[:, :w], in1=bp_rep[:, lo:lo + w],
                    op=mybir.AluOpType.add)
                pj_part[(b, i)] = pt
                return
            ot = np_.tile([128, 512], f32, tag="ostage", bufs=3)
            if (b, i) in pj_part:
                nc.vector.tensor_tensor(
                    out=ot[:, :w], in0=p[:, :w], in1=pj_part[(b, i)][:, :w],
                    op=mybir.AluOpType.add)
            else:
                nc.vector.tensor_tensor(
                    out=ot[:, :w], in0=p[:, :w], in1=bp_rep[:, lo:lo + w],
                    op=mybir.AluOpType.add)
            if final:
                q = [nc.scalar, nc.sync][fin_state["i"] % 2]
                fin_state["i"] += 1
            else:
                q = next_q()
            q.dma_start(out_d[t * 128:(t + 1) * 128, lo:lo + w], ot[:, :w])

        # software-pipelined emission
        CHAIN_ORDER = list(range(9)) + list(range(12, 20)) + [9, 10, 11]
        for i in CHAIN_ORDER[:2]:
            qkv_chain(0, i)
        brep_emit()
        for i in CHAIN_ORDER[2:]:
            qkv_chain(0, i)
        qk1 = iter(CHAIN_ORDER)
        for hp in range(6):
            attention_hp(0, hp)
            if hp % 2 == 1:
                norm_recip(0, hp // 2)
                if hp >= 3:
                    norm_apply(0, hp // 2 - 1)
            for _ in range(4 if hp < 2 else 3):
                i = next(qk1, None)
                if i is not None:
                    qkv_chain(1, i)
        norm_apply(0, 2)
        pj0 = iter(range(8))
        for hp in range(6):
            attention_hp(1, hp)
            if hp % 2 == 1:
                norm_recip(1, hp // 2)
                if hp >= 3:
                    norm_apply(1, hp // 2 - 1)
            if hp == 5:
                for i in range(8):
                    proj_chunk(1, i, 0, 4, partial=True)
            i = next(pj0, None)
            if i is not None:
                proj_chunk(0, i)
        norm_apply(1, 2)
        for i in pj0:
            proj_chunk(0, i)
        for i in range(8):
            proj_chunk(1, i, 4, KT, final=True)

    nc.compile()
    return nc


def get_compiled():
    if "nc" not in _cache:
        _cache["nc"] = _build()
    return _cache["nc"]


def make_in_maps(x, attention_mask, W_attn, b_attn, W_proj, b_proj):
    x = np.asarray(x, dtype=np.float32).astype(np.float16)
    mask = np.ascontiguousarray(
        np.asarray(attention_mask, dtype=np.float32)[:, 0, 0, :])
    wa = np.asarray(W_attn, dtype=np.float32).astype(np.float16)
    ba = np.asarray(b_attn, dtype=np.float32).astype(np.float16).reshape(1, C3)
    wp = np.asarray(W_proj, dtype=np.float32).astype(np.float16)
    bp = np.asarray(b_proj, dtype=np.float32).astype(np.float16).reshape(1, C)
    maps = []
    for i in range(N_CORES):
        maps.append({
            "x": np.ascontiguousarray(x[BC * i:BC * (i + 1)].reshape(M, C).T),
            "mask": np.ascontiguousarray(mask[BC * i:BC * (i + 1)]),
            "w_attn": wa, "b_attn": ba, "w_proj": wp, "b_proj": bp,
        })
    return maps


def kernel(x, attention_mask, W_attn, b_attn, W_proj, b_proj):
    from concourse.bass_utils import run_bass_kernel_spmd

    nc = get_compiled()
    in_maps = make_in_maps(x, attention_mask, W_attn, b_attn, W_proj, b_proj)
    last_err = None
    for _ in range(3):
        try:
            res = run_bass_kernel_spmd(nc, in_maps, list(range(N_CORES)))
            break
        except Exception as e:  # transient NRT device errors: retry
            last_err = e
    else:
        raise last_err
    out = np.concatenate(
        [res.results[i]["out"].reshape(BC, T, C) for i in range(N_CORES)], axis=0)
    return out.astype(np.float32)


# revision 16
# speedup vs baseline: 1.1435x; 1.1435x over previous
"""BERT self-attention (B=16, T=512, C=768, H=12, D=64) on 8 trn2 NeuronCores.

Data-parallel over batch: each core gets 2 batches. Matmul operands are fp16
(11-bit mantissa, ~tf32-class precision, 1 cycle/row PE streaming, FWL weight
loads); all accumulation stays fp32 in PSUM. Per core:
  xT    = x transposed on the host during input prep (fp16, [C, M] layout).
  Q^T/K^T ([feature, token] layout, lhsT = W_attn tile) and V ([token, feature]
          layout with a per-head [V_h | ones] 65-column block, lhsT = xT tile).
  S^T   = K^T-as-lhsT matmul -> scores in [key, query] layout (K=64, head pairs
          packed in PE row groups via base-partition-64 slices).
  P     = exp(S/8 + mask) on ScalarE (mask is a per-partition bias in this
          layout), written as fp16.
  y^T   = lhsT=[V_h | ones] matmul -> unnormalized y^T plus softmax row-sums as
          an extra PSUM row; row-sums are collected per batch, inverted with a
          fast Newton-Raphson reciprocal on DVE, replicated across partitions
          by small PE matmuls, and applied with a DVE multiply.
  out   = y^T-as-lhsT matmul with W_proj + b_proj (fp32 result to DRAM).
Bias adds ride the PSUM->SBUF drain (scalar activation / DVE add). Output
stores and staging DMAs round-robin across the three DMA queues.
"""

import sys

sys.path.insert(0, "/opt/trn_rl_repo")

from contextlib import ExitStack

import numpy as np

B, T, C = 16, 512, 768
H, D = 12, 64
C3 = 3 * C
N_CORES = 8
BC = B // N_CORES           # batches per core
M = BC * T                  # tokens per core
KT = C // 128               # feature k-tiles (6)
TT = M // 128               # token tiles per core (8)
NQK = 2 * C // 128          # q+k feature n-tiles (12)
VW = H * 65                 # v tile width: per-head [V_h | ones] blocks
SCALE = 1.0 / np.sqrt(D)

_cache = {}


def _build():
    import concourse.bass as bass
    import concourse.tile as tile
    from concourse import bacc, mybir
    f32 = mybir.dt.float32
    f16 = mybir.dt.float16
    Exp = mybir.ActivationFunctionType.Exp
    Ident = mybir.ActivationFunctionType.Identity

    nc = bacc.Bacc("TRN2", target_bir_lowering=False, debug=False,
                   num_devices=N_CORES)
    x_d = nc.dram_tensor("x", [C, M], f16, kind="ExternalInput").ap()
    mask_d = nc.dram_tensor("mask", [BC, T], f32, kind="ExternalInput").ap()
    wa_d = nc.dram_tensor("w_attn", [C, C3], f16, kind="ExternalInput").ap()
    ba_d = nc.dram_tensor("b_attn", [1, C3], f16, kind="ExternalInput").ap()
    wp_d = nc.dram_tensor("w_proj", [C, C], f16, kind="ExternalInput").ap()
    bp_d = nc.dram_tensor("b_proj", [1, C], f16, kind="ExternalInput").ap()
    out_d = nc.dram_tensor("out", [M, C], f32, kind="ExternalOutput").ap()

    # scalar's queue is kept free of bulk DMA: the scalar engine owns the
    # PSUM drains + EXPs and queue entries would delay them (FIFO per engine)
    dmaq2 = [nc.sync, nc.gpsimd]
    dq_state = {"i": 0}

    def next_q():
        q = dmaq2[dq_state["i"] % 2]
        dq_state["i"] += 1
        return q

    with tile.TileContext(nc) as tc, ExitStack() as ctx:
        pp = ctx.enter_context(tc.tile_pool(name="pp", bufs=1))
        np_ = ctx.enter_context(tc.tile_pool(name="norm", bufs=4))
        ap_ = ctx.enter_context(tc.tile_pool(name="att", bufs=6))
        ps_mm = ctx.enter_context(tc.tile_pool(name="ps_mm", bufs=2, space="PSUM"))
        ps_s = ctx.enter_context(tc.tile_pool(name="ps_s", bufs=2, space="PSUM"))
        ps_y = ctx.enter_context(tc.tile_pool(name="ps_y", bufs=2, space="PSUM"))

        # --- prologue -------------------------------------------------
        # Dummy activation first: forces the activation-table const DMA +
        # ACT_TABLE_LOAD to the front of the queues (lazily emitted at first
        # use, it would otherwise sit behind all the bulk input DMAs and
        # stall the first PSUM drains).
        ones = pp.tile([1, 128], f16, tag="ones")
        nc.vector.memset(ones[:], 1.0)
        ones16 = pp.tile([128, 64], f16, tag="ones16")
        nc.vector.memset(ones16[:], 1.0)
        warm = pp.tile([1, 2], f32, tag="warm")
        nc.scalar.activation(warm[:], ones[0:1, 0:2], Exp, bias=0.0, scale=1.0)

        # DMA schedule, ordered by consumption: wa[:, 0:384] (first 3 QK
        # n-tiles) and xT gate the first chains; then the rest of the QK
        # part, the V part, wp. Split across the sync and gpsimd queues.
        wa_t = [pp.tile([128, C3], f16, tag=f"wa{k}", name=f"wa{k}")
                for k in range(KT)]
        xT = [pp.tile([128, M], f16, tag=f"xT{k}", name=f"xT{k}")
              for k in range(KT)]
        for k in range(KT):
            q = nc.sync if k < 3 else nc.gpsimd
            q.dma_start(wa_t[k][:, 0:384], wa_d[k * 128:(k + 1) * 128, 0:384])
        ba_qk = pp.tile([128, NQK], f32, tag="ba_qk")
        nc.gpsimd.dma_start(
            ba_qk[:],
            ba_d[0, 0:2 * C].rearrange("(j p) -> p j", p=128))
        for k in range(KT):
            q = nc.sync if k % 2 == 0 else nc.gpsimd
            q.dma_start(xT[k][:], x_d[k * 128:(k + 1) * 128, :])
        for k in range(KT):
            q = nc.sync if k % 2 == 0 else nc.gpsimd
            q.dma_start(
                wa_t[k][:, 384:1536], wa_d[k * 128:(k + 1) * 128, 384:1536])
        mask_sb = pp.tile([128, BC * 4], f32, tag="mask")
        nc.gpsimd.dma_start(
            mask_sb[:],
            mask_d.rearrange("a b -> (a b)").rearrange("(j p) -> p j", p=128))
        ba_t = pp.tile([1, C3], f16, tag="ba")
        nc.gpsimd.dma_start(ba_t[:], ba_d[:])
        bp_t = pp.tile([1, C], f16, tag="bp")
        nc.gpsimd.dma_start(bp_t[:], bp_d[:])
        for k in range(KT):
            nc.gpsimd.dma_start(
                wa_t[k][:, 1536:2304],
                wa_d[k * 128:(k + 1) * 128, 1536:2304])
        wp_t = [pp.tile([128, C], f16, tag=f"wp{k}", name=f"wp{k}")
                for k in range(KT)]
        for k in range(KT):
            nc.sync.dma_start(wp_t[k][:], wp_d[k * 128:(k + 1) * 128, :])

        # bias rows replicated across partitions via K=1 matmuls
        ba_v_rep = pp.tile([128, C], f32, tag="ba_v_rep")
        bp_rep = pp.tile([128, C], f32, tag="bp_rep")

        def brep_emit():
            for lo, w in ((0, 512), (512, 256)):
                p = ps_mm.tile([128, 512], f32, tag="mm", name=f"brep{lo}")
                nc.tensor.matmul(
                    p[:, :w], ones[0:1, 0:128],
                    ba_t[0:1, 2 * C + lo:2 * C + lo + w], start=True, stop=True)
                nc.vector.tensor_copy(ba_v_rep[:, lo:lo + w], p[:, :w])
                p2 = ps_mm.tile([128, 512], f32, tag="mm", name=f"bprep{lo}")
                nc.tensor.matmul(
                    p2[:, :w], ones[0:1, 0:128],
                    bp_t[0:1, lo:lo + w], start=True, stop=True)
                nc.vector.tensor_copy(bp_rep[:, lo:lo + w], p2[:, :w])

        v_t = [pp.tile([128, VW], f16, tag=f"v{t}", name=f"v{t}")
               for t in range(TT)]
        qkT = [pp.tile([128, M], f16, tag=f"qk{n}", name=f"qk{n}")
               for n in range(NQK)]
        yT_t = [pp.tile([128, M], f16, tag=f"yT{c}", name=f"yT{c}")
                for c in range(KT)]
        for t in range(TT):
            nc.vector.memset(
                v_t[t].rearrange("p (h c) -> p h c", c=65)[:, :, D:D + 1], 1.0)

        def qkv_chain(b, i):
            """i in [0, 20): 12 QK n-tiles then 8 V half-tiles."""
            bcol = b * T
            if i < NQK:
                n = i
                p = ps_mm.tile([128, 512], f32, tag="mm", name=f"mm{b}_{i}")
                for k in range(KT):
                    nc.tensor.matmul(
                        p[:],
                        wa_t[k][:, n * 128:(n + 1) * 128],
                        xT[k][:, bcol:bcol + T],
                        start=(k == 0), stop=(k == KT - 1))
                # drain PSUM->SBUF fused with bias add; split scalar/DVE
                if n % 2 == 0:
                    nc.scalar.activation(
                        qkT[n][:, bcol:bcol + T], p[:], Ident,
                        bias=ba_qk[:, n:n + 1], scale=1.0)
                else:
                    nc.vector.tensor_scalar_add(
                        qkT[n][:, bcol:bcol + T], p[:], ba_qk[:, n:n + 1])
            else:
                j = i - NQK
                t = b * 4 + j // 2
                lo, w = ((0, 512), (512, 256))[j % 2]
                p = ps_mm.tile([128, 512], f32, tag="mm", name=f"mm{b}_{i}")
                for k in range(KT):
                    nc.tensor.matmul(
                        p[:, :w],
                        xT[k][:, t * 128:(t + 1) * 128],
                        wa_t[k][:, 2 * C + lo:2 * C + lo + w],
                        start=(k == 0), stop=(k == KT - 1))
                h0 = lo // D
                nc.vector.tensor_tensor(
                    out=v_t[t].rearrange("p (h c) -> p h c", c=65)
                        [:, h0:h0 + w // D, 0:D],
                    in0=p[:, :w].rearrange("p (h c) -> p h c", c=D),
                    in1=ba_v_rep[:, lo:lo + w].rearrange(
                        "p (h c) -> p h c", c=D),
                    op=mybir.AluOpType.add)

        yun_all = {}
        r_tiles = {}

        def attention_hp(b, hp):
            bcol = b * T
            if hp % 2 == 0:
                rt = np_.tile([97, 512], f32, tag="r_all", bufs=3,
                              name=f"r_all{b}_{hp // 2}")
                nc.vector.memset(rt[:], 1.0)
                r_tiles[(b, hp // 2)] = rt
            e_tiles = []
            for kt in range(4):
                ps = ps_s.tile([128, 1024], f32)
                for sub in range(2):
                    r0 = 64 * sub
                    nc.tensor.matmul(
                        ps[:, sub * 512:sub * 512 + 512],
                        qkT[6 + hp][r0:r0 + D,
                                    bcol + kt * 128:bcol + (kt + 1) * 128],
                        qkT[hp][r0:r0 + D, bcol:bcol + T],
                        start=True, stop=True)
                e = ap_.tile([128, 1024], f16, tag="e")
                nc.scalar.activation(
                    e[:], ps[:], Exp,
                    bias=mask_sb[:, b * 4 + kt:b * 4 + kt + 1],
                    scale=float(SCALE))
                e_tiles.append(e)
            for sub in range(2):
                h = 2 * hp + sub
                py = ps_y.tile([128, 512], f32)
                for kt in range(4):
                    nc.tensor.matmul(
                        py[0:65, :],
                        v_t[b * 4 + kt][:, 65 * h:65 * (h + 1)],
                        e_tiles[kt][:, sub * 512:sub * 512 + 512],
                        start=(kt == 0), stop=(kt == 3))
                yun = np_.tile([64, 512], f16, tag="yun", bufs=14,
                               name=f"yun{b}_{h}")
                nc.vector.tensor_copy(yun[:], py[0:D, :])
                rs = np_.tile([D + 1, 512], f32, tag="rstage")
                nc.vector.tensor_copy(rs[D:D + 1, :], py[D:D + 1, :])
                next_q().dma_start(
                    r_tiles[(b, hp // 2)][32 * (h % 4):32 * (h % 4) + 1, :],
                    rs[D:D + 1, :])
                yun_all[(b, h)] = yun

        recip_tiles = {}

        def norm_recip(b, grp):
            recip = np_.tile([97, 512], f32, tag="recip", bufs=3)
            nc.vector.reciprocal_approx_fast(recip[:], r_tiles[(b, grp)][:])
            recip16 = np_.tile([97, 512], f16, tag="recip16", bufs=3)
            nc.vector.tensor_copy(recip16[:], recip[:])
            recip_tiles[(b, grp)] = recip16

        def norm_apply(b, grp):
            bcol = b * T
            recip16 = recip_tiles[(b, grp)]
            # odd heads first: their cross-partition staging DMA latency is
            # hidden behind the even heads' direct multiplies
            for h in sorted(range(4 * grp, 4 * grp + 4),
                            key=lambda h: -(h % 2)):
                nt, r0 = h // 2, 64 * (h % 2)
                j = 32 * (h % 4)
                rep = ps_y.tile([128, 512], f32, tag="py", name=f"rep{b}_{h}")
                nc.tensor.matmul(
                    rep[0:64, :], ones16[j:j + 1, :], recip16[j:j + 1, :],
                    start=True, stop=True, tile_position=(j, 0))
                dst = yT_t[nt][r0:r0 + D, bcol:bcol + T]
                if r0 == 0:
                    nc.vector.tensor_mul(dst, yun_all[(b, h)][:], rep[0:64, :])
                else:
                    st = np_.tile([64, 512], f16, tag="stage")
                    nc.vector.tensor_mul(st[:], yun_all[(b, h)][:],
                                         rep[0:64, :])
                    next_q().dma_start(dst, st[:])

        pj_part = {}
        fin_state = {"i": 0}

        def proj_chunk(b, i, ks=0, ke=KT, partial=False, final=False):
            t = b * 4 + i // 2
            lo, w = ((0, 512), (512, 256))[i % 2]
            p = ps_mm.tile([128, 512], f32, tag="mm", name=f"pj{b}_{i}_{ks}")
            for k in range(ks, ke):
                nc.tensor.matmul(
                    p[:, :w],
                    yT_t[k][:, t * 128:(t + 1) * 128],
                    wp_t[k][:, lo:lo + w],
                    start=(k == ks), stop=(k == ke - 1))
            if partial:
                pt = np_.tile([128, 512], f32, tag="pjpart", bufs=8,
                              name=f"pjpart{i}")
                nc.vector.tensor_tensor(
                    out=pt[:, :w], in0=p[:, :w], in1=bp_rep[:, lo:lo + w],
                    op=mybir.AluOpType.add)
                pj_part[(b, i)] = pt
                return
            ot = np_.tile([128, 512], f32, tag="ostage", bufs=3)
            if (b, i) in pj_part:
                nc.vector.tensor_tensor(
                    out=ot[:, :w], in0=p[:, :w], in1=pj_part[(b, i)][:, :w],
                    op=mybir.AluOpType.add)
            else:
                nc.vector.tensor_tensor(
                    out=ot[:, :w], in0=p[:, :w], in1=bp_rep[:, lo:lo + w],
                    op=mybir.AluOpType.add)
            if final:
                q = [nc.scalar, nc.sync][fin_state["i"] % 2]
                fin_state["i"] += 1
            else:
                q = next_q()
            q.dma_start(out_d[t * 128:(t + 1) * 128, lo:lo + w], ot[:, :w])

        # software-pipelined emission
        CHAIN_ORDER = list(range(9)) + list(range(12, 20)) + [9, 10, 11]
        for i in CHAIN_ORDER[:2]:
            qkv_chain(0, i)
        brep_emit()
        for i in CHAIN_ORDER[2:]:
            qkv_chain(0, i)
        qk1 = iter(CHAIN_ORDER)
        for hp in range(6):
            attention_hp(0, hp)
            if hp % 2 == 1:
                norm_recip(0, hp // 2)
                if hp >= 3:
                    norm_apply(0, hp // 2 - 1)
            for _ in range(4 if hp < 2 else 3):
                i = next(qk1, None)
                if i is not None:
                    qkv_chain(1, i)
        norm_apply(0, 2)
        pj0 = iter(range(8))
        for hp in range(6):
            attention_hp(1, hp)
            if hp % 2 == 1:
                norm_recip(1, hp // 2)
                if hp >= 3:
                    norm_apply(1, hp // 2 - 1)
            if hp == 5:
                for i in range(8):
                    proj_chunk(1, i, 0, 4, partial=True)
            i = next(pj0, None)
            if i is not None:
                proj_chunk(0, i)
        norm_apply(1, 2)
        for i in pj0:
            proj_chunk(0, i)
        for i in range(8):
            proj_chunk(1, i, 4, KT, final=True)

    nc.compile()
    return nc


def get_compiled():
    if "nc" not in _cache:
        _cache["nc"] = _build()
    return _cache["nc"]


def make_in_maps(x, attention_mask, W_attn, b_attn, W_proj, b_proj):
    x = np.asarray(x, dtype=np.float32).astype(np.float16)
    mask = np.ascontiguousarray(
        np.asarray(attention_mask, dtype=np.float32)[:, 0, 0, :])
    wa = np.asarray(W_attn, dtype=np.float32).astype(np.float16)
    ba = np.asarray(b_attn, dtype=np.float32).astype(np.float16).reshape(1, C3)
    wp = np.asarray(W_proj, dtype=np.float32).astype(np.float16)
    bp = np.asarray(b_proj, dtype=np.float32).astype(np.float16).reshape(1, C)
    maps = []
    for i in range(N_CORES):
        maps.append({
            "x": np.ascontiguousarray(x[BC * i:BC * (i + 1)].reshape(M, C).T),
            "mask": np.ascontiguousarray(mask[BC * i:BC * (i + 1)]),
            "w_attn": wa, "b_attn": ba, "w_proj": wp, "b_proj": bp,
        })
    return maps


def kernel(x, attention_mask, W_attn, b_attn, W_proj, b_proj):
    from concourse.bass_utils import run_bass_kernel_spmd

    nc = get_compiled()
    in_maps = make_in_maps(x, attention_mask, W_attn, b_attn, W_proj, b_proj)
    last_err = None
    for _ in range(3):
        try:
            res = run_bass_kernel_spmd(nc, in_maps, list(range(N_CORES)))
            break
        except Exception as e:  # transient NRT device errors: retry
            last_err = e
    else:
        raise last_err
    out = np.concatenate(
        [res.results[i]["out"].reshape(BC, T, C) for i in range(N_CORES)], axis=0)
    return out.astype(np.float32)
